# revision 20
# baseline (speedup 1.0000x reference)
"""Position-only MoE router kernel for Trainium2 (8 NeuronCores, SPMD).

Problem: x[8,2048,1024], tile_sigs[8,32], W[8,1024,1024], b[8,1024].
Routing idx[s] = argmax_t( pe[s] @ sign(tile_sigs[t]) ) depends only on the
position s, so it is computed on the host and baked into the schedule at
build time.

Strategy (token-parallel, expert-sorted):
  All B*S = 16384 tokens are grouped by expert and split into 8x17 tiles of
  128 tokens. Every core runs the IDENTICAL program (required: one NEFF,
  SPMD): 17 tiles in 4 groups of (13,2,1,1) tiles; each group uses one
  expert weight slot. Which expert each group is, and which tokens each
  tile holds, is per-core DATA packed by the host. x/W/y travel as bf16
  (fp32 PSUM accumulation), so per-core HBM traffic is ~16.5 MB vs the
  ~48 MB of a batch-parallel fp32 plan.

  Bias is applied without per-tile matmuls: once per group the PE computes
  ones[128] (x) b[e] into a dedicated PSUM region, DVE replicates it to
  SBUF, and the per-tile PSUM->SBUF drain becomes a fused add
  (scalar_tensor_tensor) on DVE.

Raw Bass (no Tile framework): explicit per-engine streams + semaphores.
  SP   : xt chunk DMAs, per-tile y stores
  ACT  : bias DMA, 4 per-group W loads
  PE   : per-group bias outer product + per-tile matmuls (8 K-chunks x 2)
  DVE  : per-group bias replication, per-tile fused add PSUM->SBUF
  POOL : ones memset
"""

import math
import os
import sys

import numpy as np

for _p in ("/opt/trn_rl_repo", "/opt/trn_rl_repo/concourse"):
    if _p not in sys.path and os.path.isdir(_p):
        sys.path.append(_p)

B, S, D, T, P = 8, 2048, 1024, 8, 32
NCORES = 8
KC = D // 128  # 8 contraction chunks
NT = 17  # tiles per core (8*17*128 = 17408 slots >= 16384 tokens)
SIZES = (13, 2, 1, 1)  # group sizes (tiles); one expert weight slot each
G = len(SIZES)
PS = 3  # PSUM accumulator slots
OS = 3  # output staging slots
XCHUNKS = [(0, 1), (1, 6), (6, 11), (11, 17)]  # xt DMA chunks (tiles)

LAST_RESULTS = None  # BassKernelResults of the most recent run (for profiling)
_CACHE = {}


def _routing_idx(tile_sigs: np.ndarray) -> np.ndarray:
    pos = np.arange(S, dtype=np.float32)[:, None]
    div = np.exp(
        np.arange(0, P, 2, dtype=np.float32) * (-math.log(10000.0) / P)
    ).astype(np.float32)
    ang = pos * div
    pe = np.zeros((S, P), np.float32)
    pe[:, 0::2] = np.sin(ang)
    pe[:, 1::2] = np.cos(ang)
    scores = pe @ np.sign(tile_sigs).astype(np.float32).T
    return np.argmax(scores, axis=-1)


def _plan(idx: np.ndarray):
    """Pack expert token lists into 8 cores x groups of SIZES tiles.

    Returns per-core list of (expert, ids) where ids is an int64 array of
    length size*128 with -1 marking padding rows.
    """
    # token ids (b*S + s) per expert, position-major
    ids_by_e = []
    for e in range(T):
        pos_e = np.nonzero(idx == e)[0]
        ids = (np.arange(B, dtype=np.int64)[:, None] * S + pos_e[None, :]).ravel()
        ids_by_e.append(ids)

    # part pool: SIZES[g] appears NCORES times
    from collections import Counter

    pool = Counter()
    for s in SIZES:
        pool[s] += NCORES
    sizes_desc = sorted(pool, reverse=True)

    parts_by_size = {s: [] for s in pool}
    order = sorted(range(T), key=lambda e: -len(ids_by_e[e]))
    for e in order:
        ids = ids_by_e[e]
        off = 0
        rem = len(ids)
        while rem > 0:
            # smallest size that covers the remainder with small padding,
            # else the largest size that fits fully
            cover = [s for s in sizes_desc if pool[s] > 0 and s * 128 >= rem]
            pick = None
            if cover and (min(cover) * 128 - rem) < 256:
                pick = min(cover)
            else:
                under = [s for s in sizes_desc if pool[s] > 0 and s * 128 <= rem]
                if under:
                    pick = max(under)
                elif cover:
                    pick = min(cover)
            if pick is None:
                raise RuntimeError("infeasible part decomposition")
            take = min(rem, pick * 128)
            chunk = np.full(pick * 128, -1, dtype=np.int64)
            chunk[:take] = ids[off : off + take]
            parts_by_size[pick].append((e, chunk))
            pool[pick] -= 1
            off += take
            rem -= take
    # leftover parts = pure padding (expert 0, all -1)
    for s in sizes_desc:
        while pool[s] > 0:
            parts_by_size[s].append((0, np.full(s * 128, -1, dtype=np.int64)))
            pool[s] -= 1

    # deal parts to cores: core c takes the next unused part of each size,
    # in SIZES order (repeated sizes take successive parts)
    taken = {s: 0 for s in parts_by_size}
    cores = []
    for c in range(NCORES):
        groups = []
        for s in SIZES:
            groups.append(parts_by_size[s][taken[s]])
            taken[s] += 1
        cores.append(groups)
    return cores


def _build_nc():
    import concourse.bass as bass
    import concourse.mybir as mybir

    f32 = mybir.dt.float32
    bf16 = mybir.dt.bfloat16

    nc = bass.Bass()
    # host layouts (per core):
    #   xt [128, NT, KC, 128]  xt[p,t,k,c] = x_tok[t*128+c, k*128+p]
    #   wt [G, 128, KC, D]     wt[g,p,k,o] = W[e_g][o, k*128+p]
    #   bias [1, G*D]          bias[0, g*D+o] = b[e_g][o]
    #   y [NT*128, D]          row-major tokens
    xt_d = nc.dram_tensor("xt", [128, NT, KC, 128], bf16, kind="ExternalInput")
    # W split by output half so every W DMA is contiguous (8 KB/partition)
    wt_d = nc.dram_tensor("wt", [G, 2, 128, KC, 512], bf16, kind="ExternalInput")
    # bias replicated across partitions on the host: no PE/DVE bias setup
    br_d = nc.dram_tensor("br", [128, G, D], bf16, kind="ExternalInput")
    y_d = nc.dram_tensor("y", [NT * 128, D], bf16, kind="ExternalOutput")

    from contextlib import ExitStack

    # tile t -> group
    tile_group = []
    for g, sz in enumerate(SIZES):
        tile_group += [g] * sz

    def chunk_of(t):
        for ci, (a, bnd) in enumerate(XCHUNKS):
            if t < bnd:
                return ci
        raise AssertionError

    with ExitStack() as ctx:
        xt_sb = ctx.enter_context(nc.sbuf_tensor([128, NT, KC, 128], bf16))
        w_sb = ctx.enter_context(nc.sbuf_tensor([128, G, 2, KC, 512], bf16))
        br_sb = ctx.enter_context(nc.sbuf_tensor([128, G, D], bf16))
        out_sb = ctx.enter_context(nc.sbuf_tensor([128, OS, D], bf16))
        ps = ctx.enter_context(nc.psum_tensor([128, PS, D], f32))

        dma_x = ctx.enter_context(nc.semaphore("dma_x"))
        dma_br = ctx.enter_context(nc.semaphore("dma_br"))
        dma_w0 = ctx.enter_context(nc.semaphore("dma_w0"))
        dma_w = ctx.enter_context(nc.semaphore("dma_w"))
        dma_y_s = [
            ctx.enter_context(nc.semaphore(f"dma_y{i}")) for i in range(OS)
        ]
        pe_h = ctx.enter_context(nc.semaphore("pe_h"))
        dve_c = ctx.enter_context(nc.semaphore("dve_c"))
        block = ctx.enter_context(nc.Block())

        y_count = [len(range(s, NT, OS)) for s in range(OS)]

        # SP carries everything latency-critical, ALONE on its queue early
        # (the ACT queue wins ring arbitration and would starve it): W0 in
        # four contiguous 0.5 MB pieces interleaved with the first tile,
        # the bias, the remaining xt chunks, then all y stores.
        @block.sync
        def _(eng):
            eng.dma_start(w_sb[:, 0, 0, 0:4, :], wt_d[0, 0, :, 0:4, :]).then_inc(
                dma_w0, 16
            )
            a, bnd = XCHUNKS[0]
            eng.dma_start(
                xt_sb[:, a:bnd, :, :], xt_d[:, a:bnd, :, :]
            ).then_inc(dma_x, 16)
            eng.dma_start(w_sb[:, 0, 0, 4:8, :], wt_d[0, 0, :, 4:8, :]).then_inc(
                dma_w0, 16
            )
            eng.dma_start(w_sb[:, 0, 1, 0:4, :], wt_d[0, 1, :, 0:4, :]).then_inc(
                dma_w0, 16
            )
            eng.dma_start(w_sb[:, 0, 1, 4:8, :], wt_d[0, 1, :, 4:8, :]).then_inc(
                dma_w0, 16
            )
            eng.dma_start(br_sb[:], br_d[:]).then_inc(dma_br, 16)
            for a, bnd in XCHUNKS[1:]:
                eng.dma_start(
                    xt_sb[:, a:bnd, :, :], xt_d[:, a:bnd, :, :]
                ).then_inc(dma_x, 16)
            for t in range(NT):
                eng.wait_ge(dve_c, t + 1)
                eng.dma_start(
                    y_d[t * 128 : (t + 1) * 128, :], out_sb[:, t % OS, :]
                ).then_inc(dma_y_s[t % OS], 16)
            for s in range(OS):
                eng.wait_ge(dma_y_s[s], 16 * y_count[s])

        # ACT: hold the big W1-3 loads until the xt chunks are in so they
        # don't contend for the early window (needed from tile 13 only).
        @block.scalar
        def _(eng):
            eng.wait_ge(dma_x, 16 * len(XCHUNKS))
            for g in range(1, G):
                for h in range(2):
                    eng.dma_start(w_sb[:, g, h, :, :], wt_d[g, h]).then_inc(
                        dma_w, 16
                    )

        @block.tensor
        def _(eng):
            # warm-up: dummy matmuls on not-yet-loaded SBUF (tile 16 /
            # W slot 3 arrive much later; tile 0 h0 overwrites the psum
            # with start=True) so HAM is un-throttled when tile 0 starts.
            for _i in range(10):
                eng.matmul(
                    ps[:, 0, 0:512],
                    xt_sb[:, NT - 1, 0, :],
                    w_sb[:, G - 1, 1, 0, :],
                    start=True,
                    stop=True,
                )
            t = 0
            last_chunk = -1
            w0_seen = 0
            for g in range(G):
                for _i in range(SIZES[g]):
                    first = _i == 0
                    c = chunk_of(t)
                    if c > last_chunk:
                        eng.wait_ge(dma_x, 16 * (c + 1))
                        last_chunk = c
                    if g >= 1 and first:
                        eng.wait_ge(dma_w, 32 * g)
                    if t >= PS:
                        eng.wait_ge(dve_c, t - PS + 1)
                    for h in range(2):
                        for k in range(KC):
                            if g == 0 and w0_seen < 16 * (2 * h + k // 4 + 1):
                                w0_seen = 16 * (2 * h + k // 4 + 1)
                                eng.wait_ge(dma_w0, w0_seen)
                            mm = eng.matmul(
                                ps[:, t % PS, h * 512 : (h + 1) * 512],
                                xt_sb[:, t, k, :],
                                w_sb[:, g, h, k, :],
                                start=(k == 0),
                                stop=(k == KC - 1),
                            )
                        mm.then_inc(pe_h, 1)
                    t += 1

        @block.vector
        def _(eng):
            eng.wait_ge(dma_br, 16)
            t = 0
            for g in range(G):
                for _i in range(SIZES[g]):
                    if t >= OS:
                        eng.wait_ge(dma_y_s[t % OS], 16 * (t // OS))
                    for h in range(2):
                        eng.wait_ge(pe_h, 2 * t + h + 1)
                        stt = eng.scalar_tensor_tensor(
                            out_sb[:, t % OS, h * 512 : (h + 1) * 512],
                            ps[:, t % PS, h * 512 : (h + 1) * 512],
                            0.0,
                            br_sb[:, g, h * 512 : (h + 1) * 512],
                            op0=mybir.AluOpType.add,
                            op1=mybir.AluOpType.add,
                        )
                    stt.then_inc(dve_c, 1)
                    t += 1

    return nc


def kernel(x, tile_sigs, W, b):
    global LAST_RESULTS
    import ml_dtypes
    from concourse.bass_utils import run_bass_kernel_spmd

    bf16 = ml_dtypes.bfloat16

    x = np.asarray(x, dtype=np.float32)
    tile_sigs = np.asarray(tile_sigs, dtype=np.float32)
    W = np.asarray(W, dtype=np.float32)
    b = np.asarray(b, dtype=np.float32)

    idx = _routing_idx(tile_sigs)
    cores = _plan(idx)

    key = ("v5", NT, SIZES)
    if key in _CACHE:
        nc = _CACHE[key]
    else:
        nc = _build_nc()
        _CACHE[key] = nc

    # host-side shard prep (all bf16)
    xflat = np.ascontiguousarray(x.reshape(B * S, D)).astype(bf16)
    # wt_all[e][h,p,k,c] = W[e][h*512+c, k*128+p]
    wt_all = np.ascontiguousarray(
        W.transpose(0, 2, 1)
        .reshape(T, KC, 128, 2, 512)
        .transpose(0, 3, 2, 1, 4)
    ).astype(bf16)
    b_bf = b.astype(bf16)

    in_maps = []
    ids_per_core = []
    for c in range(NCORES):
        groups = cores[c]
        ids = np.concatenate([g[1] for g in groups])  # [NT*128]
        ids_per_core.append(ids)
        safe = np.where(ids < 0, 0, ids)
        xg = xflat[safe]  # [NT*128, D] bf16
        xg[ids < 0] = 0
        xt = np.ascontiguousarray(
            xg.reshape(NT, 128, KC, 128).transpose(3, 0, 2, 1)
        )  # [128, NT, KC, 128]
        wt = np.ascontiguousarray(
            np.stack([wt_all[e] for e, _ in groups])
        )  # [G, 2, 128, KC, 512]
        br = np.ascontiguousarray(
            np.broadcast_to(
                np.stack([b_bf[e] for e, _ in groups])[None, :, :], (128, G, D)
            )
        )
        in_maps.append({"xt": xt, "wt": wt, "br": br})

    core_ids = list(range(NCORES))
    res = run_bass_kernel_spmd(nc, in_maps, core_ids)
    LAST_RESULTS = res

    out = np.empty((B * S, D), dtype=np.float32)
    for c in range(NCORES):
        yp = res.results[c]["y"]  # [NT*128, D] bf16
        ids = ids_per_core[c]
        valid = ids >= 0
        out[ids[valid]] = yp[valid].astype(np.float32)
    return out.reshape(B, S, D)


# revision 25
# speedup vs baseline: 1.0779x; 1.0779x over previous
"""Position-only MoE router kernel for Trainium2 (8 NeuronCores, SPMD).

Problem: x[8,2048,1024], tile_sigs[8,32], W[8,1024,1024], b[8,1024].
Routing idx[s] = argmax_t( pe[s] @ sign(tile_sigs[t]) ) depends only on the
position s, so it is computed on the host and baked into the schedule at
build time.

Strategy (token-parallel, expert-sorted):
  All B*S = 16384 tokens are grouped by expert and split into 8x17 tiles of
  128 tokens. Every core runs the IDENTICAL program (required: one NEFF,
  SPMD): 17 tiles in 4 groups of (13,2,1,1) tiles; each group uses one
  expert weight slot. Which expert each group is, and which tokens each
  tile holds, is per-core DATA packed by the host. x/W/y travel as bf16
  (fp32 PSUM accumulation), so per-core HBM traffic is ~16.5 MB vs the
  ~48 MB of a batch-parallel fp32 plan.

  Bias is applied without per-tile matmuls: once per group the PE computes
  ones[128] (x) b[e] into a dedicated PSUM region, DVE replicates it to
  SBUF, and the per-tile PSUM->SBUF drain becomes a fused add
  (scalar_tensor_tensor) on DVE.

Raw Bass (no Tile framework): explicit per-engine streams + semaphores.
  SP   : xt chunk DMAs, per-tile y stores
  ACT  : bias DMA, 4 per-group W loads
  PE   : per-group bias outer product + per-tile matmuls (8 K-chunks x 2)
  DVE  : per-group bias replication, per-tile fused add PSUM->SBUF
  POOL : ones memset
"""

import math
import os
import sys

import numpy as np

for _p in ("/opt/trn_rl_repo", "/opt/trn_rl_repo/concourse"):
    if _p not in sys.path and os.path.isdir(_p):
        sys.path.append(_p)

B, S, D, T, P = 8, 2048, 1024, 8, 32
NCORES = 8
KC = D // 128  # 8 contraction chunks
NT = 17  # tiles per core (8*17*128 = 17408 slots >= 16384 tokens)
SIZES = (13, 2, 1, 1)  # group sizes (tiles); one expert weight slot each
G = len(SIZES)
PS = 3  # PSUM accumulator slots
OS = 3  # output staging slots
XCHUNKS = [(0, 2), (2, 6), (6, 11), (11, 17)]  # xt DMA chunks (tiles)

LAST_RESULTS = None  # BassKernelResults of the most recent run (for profiling)
_CACHE = {}


def _routing_idx(tile_sigs: np.ndarray) -> np.ndarray:
    pos = np.arange(S, dtype=np.float32)[:, None]
    div = np.exp(
        np.arange(0, P, 2, dtype=np.float32) * (-math.log(10000.0) / P)
    ).astype(np.float32)
    ang = pos * div
    pe = np.zeros((S, P), np.float32)
    pe[:, 0::2] = np.sin(ang)
    pe[:, 1::2] = np.cos(ang)
    scores = pe @ np.sign(tile_sigs).astype(np.float32).T
    return np.argmax(scores, axis=-1)


def _try_plan(idx: np.ndarray, sizes):
    """Pack expert token lists into 8 cores x groups of `sizes` tiles.

    Returns per-core list of (expert, ids) where ids is an int64 array of
    length size*128 with -1 marking padding rows, or None if infeasible.
    """
    # token ids (b*S + s) per expert, position-major
    ids_by_e = []
    for e in range(T):
        pos_e = np.nonzero(idx == e)[0]
        ids = (np.arange(B, dtype=np.int64)[:, None] * S + pos_e[None, :]).ravel()
        ids_by_e.append(ids)

    # part pool: sizes[g] appears NCORES times
    from collections import Counter

    pool = Counter()
    for s in sizes:
        pool[s] += NCORES
    sizes_desc = sorted(pool, reverse=True)

    parts_by_size = {s: [] for s in pool}
    order = sorted(range(T), key=lambda e: -len(ids_by_e[e]))
    for e in order:
        ids = ids_by_e[e]
        off = 0
        rem = len(ids)
        while rem > 0:
            # smallest size that covers the remainder with small padding,
            # else the largest size that fits fully
            cover = [s for s in sizes_desc if pool[s] > 0 and s * 128 >= rem]
            pick = None
            if cover and (min(cover) * 128 - rem) < 256:
                pick = min(cover)
            else:
                under = [s for s in sizes_desc if pool[s] > 0 and s * 128 <= rem]
                if under:
                    pick = max(under)
                elif cover:
                    pick = min(cover)
            if pick is None:
                return None
            take = min(rem, pick * 128)
            chunk = np.full(pick * 128, -1, dtype=np.int64)
            chunk[:take] = ids[off : off + take]
            parts_by_size[pick].append((e, chunk))
            pool[pick] -= 1
            off += take
            rem -= take
    # leftover parts = pure padding (expert 0, all -1)
    for s in sizes_desc:
        while pool[s] > 0:
            parts_by_size[s].append((0, np.full(s * 128, -1, dtype=np.int64)))
            pool[s] -= 1

    # deal parts to cores: core c takes the next unused part of each size,
    # in sizes order (repeated sizes take successive parts)
    taken = {s: 0 for s in parts_by_size}
    cores = []
    for c in range(NCORES):
        groups = []
        for s in sizes:
            groups.append(parts_by_size[s][taken[s]])
            taken[s] += 1
        cores.append(groups)
    return cores


def _plan(idx: np.ndarray):
    """Find a feasible uniform (sizes, plan); grow NT if needed."""
    cand = [SIZES]
    for nt in range(NT + 1, NT + 8):
        cand.append((nt - 4, 2, 1, 1))
        cand.append((nt - 5, 3, 1, 1))
        cand.append((nt - 6, 2, 2, 2))
    for sizes in cand:
        cores = _try_plan(idx, sizes)
        if cores is not None:
            return sizes, cores
    raise RuntimeError("no feasible uniform plan found")


def _build_nc():
    import concourse.bass as bass
    import concourse.mybir as mybir

    f32 = mybir.dt.float32
    bf16 = mybir.dt.bfloat16

    nc = bass.Bass()
    # host layouts (per core):
    #   xt [128, NT, KC, 128]  xt[p,t,k,c] = x_tok[t*128+c, k*128+p]
    #   wt [G, 128, KC, D]     wt[g,p,k,o] = W[e_g][o, k*128+p]
    #   bias [1, G*D]          bias[0, g*D+o] = b[e_g][o]
    #   y [NT*128, D]          row-major tokens
    xt_d = nc.dram_tensor("xt", [128, NT, KC, 128], bf16, kind="ExternalInput")
    # W split by output half so every W DMA is contiguous (8 KB/partition)
    wt_d = nc.dram_tensor("wt", [G, 2, 128, KC, 512], bf16, kind="ExternalInput")
    # bias replicated across partitions on the host: no PE/DVE bias setup
    br_d = nc.dram_tensor("br", [128, G, D], bf16, kind="ExternalInput")
    y_d = nc.dram_tensor("y", [NT * 128, D], bf16, kind="ExternalOutput")

    from contextlib import ExitStack

    # tile t -> group
    tile_group = []
    for g, sz in enumerate(SIZES):
        tile_group += [g] * sz

    def chunk_of(t):
        for ci, (a, bnd) in enumerate(XCHUNKS):
            if t < bnd:
                return ci
        raise AssertionError

    with ExitStack() as ctx:
        xt_sb = ctx.enter_context(nc.sbuf_tensor([128, NT, KC, 128], bf16))
        w_sb = ctx.enter_context(nc.sbuf_tensor([128, G, 2, KC, 512], bf16))
        br_sb = ctx.enter_context(nc.sbuf_tensor([128, G, D], bf16))
        out_sb = ctx.enter_context(nc.sbuf_tensor([128, OS, D], bf16))
        ps = ctx.enter_context(nc.psum_tensor([128, PS, D], f32))

        dma_x = ctx.enter_context(nc.semaphore("dma_x"))
        dma_br = ctx.enter_context(nc.semaphore("dma_br"))
        dma_w0 = ctx.enter_context(nc.semaphore("dma_w0"))
        dma_w = ctx.enter_context(nc.semaphore("dma_w"))
        dma_y_s = [
            ctx.enter_context(nc.semaphore(f"dma_y{i}")) for i in range(OS)
        ]
        pe_h = ctx.enter_context(nc.semaphore("pe_h"))
        dve_c = ctx.enter_context(nc.semaphore("dve_c"))
        block = ctx.enter_context(nc.Block())

        y_count = [len(range(s, NT, OS)) for s in range(OS)]

        # SP carries everything latency-critical, ALONE on its queue early
        # (the ACT queue wins ring arbitration and would starve it): W0 in
        # four contiguous 0.5 MB pieces interleaved with the first tile,
        # the bias, the remaining xt chunks, then all y stores.
        @block.sync
        def _(eng):
            eng.dma_start(w_sb[:, 0, 0, 0:4, :], wt_d[0, 0, :, 0:4, :]).then_inc(
                dma_w0, 16
            )
            a, bnd = XCHUNKS[0]
            eng.dma_start(
                xt_sb[:, a:bnd, :, :], xt_d[:, a:bnd, :, :]
            ).then_inc(dma_x, 16)
            eng.dma_start(w_sb[:, 0, 0, 4:8, :], wt_d[0, 0, :, 4:8, :]).then_inc(
                dma_w0, 16
            )
            eng.dma_start(w_sb[:, 0, 1, 0:4, :], wt_d[0, 1, :, 0:4, :]).then_inc(
                dma_w0, 16
            )
            eng.dma_start(w_sb[:, 0, 1, 4:8, :], wt_d[0, 1, :, 4:8, :]).then_inc(
                dma_w0, 16
            )
            a, bnd = XCHUNKS[1]
            eng.dma_start(
                xt_sb[:, a:bnd, :, :], xt_d[:, a:bnd, :, :]
            ).then_inc(dma_x, 16)
            eng.dma_start(br_sb[:], br_d[:]).then_inc(dma_br, 16)
            for a, bnd in XCHUNKS[2:]:
                eng.dma_start(
                    xt_sb[:, a:bnd, :, :], xt_d[:, a:bnd, :, :]
                ).then_inc(dma_x, 16)
            for t in range(NT):
                eng.wait_ge(dve_c, t + 1)
                eng.dma_start(
                    y_d[t * 128 : (t + 1) * 128, :], out_sb[:, t % OS, :]
                ).then_inc(dma_y_s[t % OS], 16)
            for s in range(OS):
                eng.wait_ge(dma_y_s[s], 16 * y_count[s])

        # ACT: hold the big W1-3 loads until the xt chunks are in so they
        # don't contend for the early window (needed from tile 13 only).
        @block.scalar
        def _(eng):
            eng.wait_ge(dma_x, 16 * len(XCHUNKS))
            for g in range(1, G):
                for h in range(2):
                    eng.dma_start(w_sb[:, g, h, :, :], wt_d[g, h]).then_inc(
                        dma_w, 16
                    )

        @block.tensor
        def _(eng):
            # warm-up: dummy matmuls on not-yet-loaded SBUF (tile 16 /
            # W slot 3 arrive much later; tile 0 h0 overwrites the psum
            # with start=True) so HAM is un-throttled when tile 0 starts.
            for _i in range(10):
                eng.matmul(
                    ps[:, 0, 0:512],
                    xt_sb[:, NT - 1, 0, :],
                    w_sb[:, G - 1, 1, 0, :],
                    start=True,
                    stop=True,
                )
            t = 0
            last_chunk = -1
            w0_seen = 0
            for g in range(G):
                for _i in range(SIZES[g]):
                    first = _i == 0
                    c = chunk_of(t)
                    if c > last_chunk:
                        eng.wait_ge(dma_x, 16 * (c + 1))
                        last_chunk = c
                    if g >= 1 and first:
                        eng.wait_ge(dma_w, 32 * g)
                    if t >= PS:
                        eng.wait_ge(dve_c, t - PS + 1)
                    for h in range(2):
                        for k in range(KC):
                            if g == 0 and w0_seen < 16 * (2 * h + k // 4 + 1):
                                w0_seen = 16 * (2 * h + k // 4 + 1)
                                eng.wait_ge(dma_w0, w0_seen)
                            mm = eng.matmul(
                                ps[:, t % PS, h * 512 : (h + 1) * 512],
                                xt_sb[:, t, k, :],
                                w_sb[:, g, h, k, :],
                                start=(k == 0),
                                stop=(k == KC - 1),
                            )
                        mm.then_inc(pe_h, 1)
                    t += 1

        @block.vector
        def _(eng):
            eng.wait_ge(dma_br, 16)
            t = 0
            for g in range(G):
                for _i in range(SIZES[g]):
                    if t >= OS:
                        eng.wait_ge(dma_y_s[t % OS], 16 * (t // OS))
                    for h in range(2):
                        eng.wait_ge(pe_h, 2 * t + h + 1)
                        stt = eng.scalar_tensor_tensor(
                            out_sb[:, t % OS, h * 512 : (h + 1) * 512],
                            ps[:, t % PS, h * 512 : (h + 1) * 512],
                            0.0,
                            br_sb[:, g, h * 512 : (h + 1) * 512],
                            op0=mybir.AluOpType.add,
                            op1=mybir.AluOpType.add,
                        )
                    stt.then_inc(dve_c, 1)
                    t += 1

    return nc


def kernel(x, tile_sigs, W, b):
    global LAST_RESULTS
    import ml_dtypes
    from concourse.bass_utils import run_bass_kernel_spmd

    bf16 = ml_dtypes.bfloat16

    x = np.asarray(x, dtype=np.float32)
    tile_sigs = np.asarray(tile_sigs, dtype=np.float32)
    W = np.asarray(W, dtype=np.float32)
    b = np.asarray(b, dtype=np.float32)

    idx = _routing_idx(tile_sigs)
    cores = _plan(idx)

    key = ("v6", NT, SIZES)
    if key in _CACHE:
        nc = _CACHE[key]
    else:
        nc = _build_nc()
        _CACHE[key] = nc

    # host-side shard prep (all bf16)
    xflat = np.ascontiguousarray(x.reshape(B * S, D)).astype(bf16)
    # wt_all[e][h,p,k,c] = W[e][h*512+c, k*128+p]
    wt_all = np.ascontiguousarray(
        W.transpose(0, 2, 1)
        .reshape(T, KC, 128, 2, 512)
        .transpose(0, 3, 2, 1, 4)
    ).astype(bf16)
    b_bf = b.astype(bf16)

    in_maps = []
    ids_per_core = []
    for c in range(NCORES):
        groups = cores[c]
        ids = np.concatenate([g[1] for g in groups])  # [NT*128]
        ids_per_core.append(ids)
        safe = np.where(ids < 0, 0, ids)
        xg = xflat[safe]  # [NT*128, D] bf16
        xg[ids < 0] = 0
        xt = np.ascontiguousarray(
            xg.reshape(NT, 128, KC, 128).transpose(3, 0, 2, 1)
        )  # [128, NT, KC, 128]
        wt = np.ascontiguousarray(
            np.stack([wt_all[e] for e, _ in groups])
        )  # [G, 2, 128, KC, 512]
        br = np.ascontiguousarray(
            np.broadcast_to(
                np.stack([b_bf[e] for e, _ in groups])[None, :, :], (128, G, D)
            )
        )
        in_maps.append({"xt": xt, "wt": wt, "br": br})

    core_ids = list(range(NCORES))
    res = run_bass_kernel_spmd(nc, in_maps, core_ids)
    LAST_RESULTS = res

    out = np.empty((B * S, D), dtype=np.float32)
    for c in range(NCORES):
        yp = res.results[c]["y"]  # [NT*128, D] bf16
        ids = ids_per_core[c]
        valid = ids >= 0
        out[ids[valid]] = yp[valid].astype(np.float32)
    return out.reshape(B, S, D)
